# revision 1
# baseline (speedup 1.0000x reference)
"""GCN message-passing + FFN kernel for Trainium2 (8 NeuronCores).

Strategy (dst-sharded, zero collectives, bf16 datapath):
  - Sort edges by dst on host (index-only prep), pad nodes to 50176 = 8*49*128.
  - Core c owns dst rows [c*6272, (c+1)*6272): it processes every edge whose
    dst lands in its range, so partial aggregates never cross cores.
  - Degree factors are folded on host: the gather table is
    x~ = x / sqrt(deg) in bf16, and the per-edge weight is
    w~ = ew / sqrt(deg[dst]) in bf16 — no device prepass.
  - Per dst-block of 128 nodes: gather x~[src] rows with dma_gather (bf16,
    256B rows; int16 indices force a lo/hi table split at 32768), build a
    scaled one-hot mask [edge, dst_local] = w~[e] * (dstl[e]==q) in bf16 with
    one fused DVE tensor_scalar(is_equal, mult) (2x 16-bit mode), and
    matmul-accumulate aggT[feat, dst] in PSUM over the block's chunks
    (bf16 matmul: 1 PE cycle/row vs 4 for fp32).
  - Blocks are gathered in groups of G=2 (contiguous idx columns), one lo +
    one hi dma_gather per group. Masks are split ~50/50 between DVE (iota
    read from PSUM to stay off the SBUF ports the gathers need) and ACT
    (2 ops, private SBUF ports) — both measured necessary on HW.
  - Per-block chunk counts are the max over the 8 cores for that block slot
    (SPMD: one program, per-core data), minimizing padding.
  - FFN fused per block in bf16: hT = relu(W1.T @ aggT + b1),
    out = hT.T @ W2 (+b2), direct DMA of the [128,128] f32 output block.
"""
import sys

sys.path.insert(0, "/opt/trn_rl_repo")

import numpy as np
import ml_dtypes

import concourse.bacc as bacc
import concourse.mybir as mybir
import concourse.tile as tile
from concourse.bass_utils import run_bass_kernel_spmd

P = 128
D = 128
NCORES = 8
N_NODES = 50000
NPAD = 50176          # next multiple of 128*8 above 50000
NBLK = NPAD // P      # 392 blocks
NBC = NBLK // NCORES  # 49 blocks per core
HALF = 32768          # int16 index limit for dma_gather

f32 = mybir.dt.float32
bf16 = mybir.dt.bfloat16
i16 = mybir.dt.int16
np_bf16 = np.dtype(ml_dtypes.bfloat16)


def _host_pack(x, src, dst, edge_weights, split=HALF, negpad=False,
               keep0_blocks=8, stream_masks=False):
    """Index-only host prep: degree-folded tables, dst-sort, lo/hi split by
    src, per-core columnar edge metadata + wrapped int16 gather indices.
    split: src value separating the lo/hi gather halves (HALF for the plain
    [NPAD, D] table; NPAD//2 for the 512B-row packed table)."""
    E = src.shape[0]
    src = np.asarray(src).astype(np.int64)
    dst = np.asarray(dst).astype(np.int64)
    ew = np.asarray(edge_weights).astype(np.float32)

    deg = np.bincount(src, minlength=NPAD)
    deg = np.maximum(deg, 1).astype(np.float32)  # exact ints, <=2^24

    order = np.argsort(dst, kind="stable")
    ds = dst[order]
    ss = src[order]
    es = ew[order]

    g = ds >> 7                                   # global block id (dst-sorted)
    ishi = (ss >= split).astype(np.int64)
    key = g * 2 + ishi                            # lo edges first within block
    order2 = np.argsort(key, kind="stable")
    ds, ss, es, g, ishi, key = (a[order2] for a in (ds, ss, es, g, ishi, key))

    kcounts = np.bincount(key, minlength=NBLK * 2)
    lo_cnt = kcounts[0::2].reshape(NCORES, NBC)   # [core, slot]
    hi_cnt = kcounts[1::2].reshape(NCORES, NBC)
    # per-slot chunk counts = max over cores (SPMD single program)
    C_lo_s = np.maximum(1, np.ceil(lo_cnt.max(axis=0) / P).astype(int))
    C_hi_s = np.maximum(1, np.ceil(hi_cnt.max(axis=0) / P).astype(int))
    C_s = C_lo_s + C_hi_s
    col_off = np.concatenate([[0], np.cumsum(C_s)[:-1]])      # chunk col base
    M = int(C_s.sum())

    kstarts = np.concatenate([[0], np.cumsum(kcounts)[:-1]])
    rank = np.arange(E) - kstarts[key]            # rank within (block, lo/hi)
    b_loc = g % NBC
    slot = rank + ishi * (C_lo_s[b_loc] * P)      # slot within block
    p_lane = slot % P
    t_chunk = slot // P
    core = g // NBC
    col = col_off[b_loc] + t_chunk

    wt = es / np.sqrt(deg[ds])                    # w~ = ew / sqrt(deg_dst)

    dstl_all = np.zeros((NCORES, P, M), np.float32)
    wt_all = np.zeros((NCORES, P, M), np.float32)
    dstl_all[core, p_lane, col] = (ds & 127).astype(np.float32)
    wt_all[core, p_lane, col] = wt
    mask_all = None
    if stream_masks:
        # fully materialized scaled one-hot masks, streamed from HBM on
        # device instead of being built by DVE/ACT ops
        mask_all = np.zeros((NCORES, P, M, P), np_bf16)
        mask_all[core, p_lane, col, ds & 127] = wt.astype(np_bf16)

    # wrapped int16 gather index arrays: per (core, block-slot), gather slot i
    # lives at [i % 16, s_off + i//16]; replicated across the 8 Q7 groups.
    def build_idx(nchunk_s, sel, values, slots):
        s_off16 = np.concatenate([[0], np.cumsum(nchunk_s * 8)[:-1]])
        ST = int((nchunk_s * 8).sum())            # int16 cols total
        if negpad:
            # padding slots get idx -1: the gather ucode skips trailing
            # negative indices (no descriptor, no HBM read). Blocks whose
            # gather buffer sees its first use keep idx 0 so the buffer
            # region stays finite (later uses read stale-but-finite rows).
            arr = np.full((NCORES, 16, ST), -1, np.int16)
            k0 = int(s_off16[keep0_blocks]) if keep0_blocks < NBC else ST
            arr[:, :, :k0] = 0
        else:
            arr = np.zeros((NCORES, 16, ST), np.int16)
        c, b, sl, v = core[sel], b_loc[sel], slots[sel], values[sel]
        arr[c, sl % 16, s_off16[b] + sl // 16] = v.astype(np.int16)
        return np.tile(arr, (1, 8, 1)), s_off16, ST

    is_lo = ishi == 0
    ilo16, lo_off16, ST_lo = build_idx(C_lo_s, is_lo, ss, slot)
    ihi16, hi_off16, ST_hi = build_idx(C_hi_s, ~is_lo, ss - split,
                                       slot - C_lo_s[b_loc] * P)

    xpad = np.zeros((NPAD, D), np.float32)
    xpad[:N_NODES] = np.asarray(x, dtype=np.float32)
    xpad /= np.sqrt(deg)[:, None]                 # x~ = x / sqrt(deg)
    xpad16 = xpad.astype(np_bf16)
    if split == NPAD // 2:
        # packed table: row r = [x~(r), x~(r + NPAD//2)] -> 512B descriptors
        xpad16 = np.concatenate([xpad16[:split], xpad16[split:]], axis=1)
    # scalar operands (per-partition columns) must be f32 for the DVE op
    wt16 = wt_all
    dstl16 = dstl_all

    layout = dict(C_lo_s=C_lo_s.tolist(), C_hi_s=C_hi_s.tolist(),
                  col_off=col_off.tolist(), M=M,
                  lo_off16=lo_off16.tolist(), hi_off16=hi_off16.tolist(),
                  ST_lo=ST_lo, ST_hi=ST_hi)
    return layout, xpad16, ilo16, ihi16, dstl16, wt16, mask_all


def _build_program(layout, b2_nonzero, repeats=1, gat_bufs=6, mask_bufs=16,
                   act_mask=lambda t: t % 2 == 1, G=2, agg_bufs=2,
                   no_gather=False, no_mask=False,
                   iota_bf16_ps=False, scalars_ps=False,
                   pack512=False, sp=False, stream_masks=False):
    C_lo_s, C_hi_s = layout["C_lo_s"], layout["C_hi_s"]
    col_off, M = layout["col_off"], layout["M"]
    lo_off16, hi_off16 = layout["lo_off16"], layout["hi_off16"]
    ST_lo, ST_hi = layout["ST_lo"], layout["ST_hi"]
    nc = bacc.Bacc("TRN2", target_bir_lowering=False, debug=False,
                   num_swdge_queues=4)

    TW = 2 * D if pack512 else D              # table row width (elements)
    TH = NPAD // 2 if pack512 else NPAD       # table rows
    SPL = NPAD // 2 if pack512 else HALF      # lo/hi split point
    xt = nc.dram_tensor("xt", [TH, TW], bf16, kind="ExternalInput")
    ilo_d = nc.dram_tensor("ilo", [P, ST_lo], i16, kind="ExternalInput")
    ihi_d = nc.dram_tensor("ihi", [P, ST_hi], i16, kind="ExternalInput")
    dstl_d = nc.dram_tensor("dstl", [P, M], f32, kind="ExternalInput")
    wt_d = nc.dram_tensor("wt", [P, M], f32, kind="ExternalInput")
    if act_mask is not None:
        ndstl_d = nc.dram_tensor("ndstl", [P, M], f32, kind="ExternalInput")
        nwt_d = nc.dram_tensor("nwt", [P, M], f32, kind="ExternalInput")
    if stream_masks:
        maskt_d = nc.dram_tensor("maskt", [P, M * P], bf16,
                                 kind="ExternalInput")
    iota_d = nc.dram_tensor("iota", [P, P], bf16, kind="ExternalInput")
    w1_d = nc.dram_tensor("w1", [D, D], bf16, kind="ExternalInput")
    w2_d = nc.dram_tensor("w2", [D, D], bf16, kind="ExternalInput")
    b1_d = nc.dram_tensor("b1", [D, 1], f32, kind="ExternalInput")
    if b2_nonzero:
        b2b_d = nc.dram_tensor("b2b", [P, D], f32, kind="ExternalInput")
    out_d = nc.dram_tensor("out", [NBC * P, D], f32, kind="ExternalOutput")

    with tile.TileContext(nc) as tc:
        with tc.tile_pool(name="meta", bufs=1) as meta, \
             tc.tile_pool(name="gat", bufs=gat_bufs) as gat, \
             tc.tile_pool(name="gath", bufs=gat_bufs) as gath, \
             tc.tile_pool(name="msk", bufs=mask_bufs) as msk, \
             tc.tile_pool(name="eptp", bufs=3) as eptp, \
             tc.tile_pool(name="ps_agg", bufs=agg_bufs, space="PSUM") as ps_agg, \
             tc.tile_pool(name="ps_h", bufs=2, space="PSUM") as ps_h, \
             tc.tile_pool(name="ps_o", bufs=2, space="PSUM") as ps_o, \
             tc.tile_pool(name="ps_c", bufs=1, space="PSUM") as ps_c:

            SL0 = max(8, (ST_lo // 8) & ~7)
            SH0 = max(8, (ST_hi // 8) & ~7)
            M0 = max(8, M // 8)
            ilo_sb = meta.tile([P, ST_lo], i16)
            nc.sync.dma_start(out=ilo_sb[:, 0:SL0], in_=ilo_d.ap()[:, 0:SL0])
            ihi_sb = meta.tile([P, ST_hi], i16)
            nc.sync.dma_start(out=ihi_sb[:, 0:SH0], in_=ihi_d.ap()[:, 0:SH0])
            dstl_sb = meta.tile([P, M], f32)
            nc.sync.dma_start(out=dstl_sb[:, 0:M0], in_=dstl_d.ap()[:, 0:M0])
            wt_sb = meta.tile([P, M], f32)
            nc.sync.dma_start(out=wt_sb[:, 0:M0], in_=wt_d.ap()[:, 0:M0])
            if act_mask is not None:
                ndstl_sb = meta.tile([P, M], f32)
                nc.sync.dma_start(out=ndstl_sb[:], in_=ndstl_d.ap())
                nwt_sb = meta.tile([P, M], f32)
                nc.sync.dma_start(out=nwt_sb[:], in_=nwt_d.ap())
            nc.sync.dma_start(out=ilo_sb[:, SL0:], in_=ilo_d.ap()[:, SL0:])
            nc.sync.dma_start(out=ihi_sb[:, SH0:], in_=ihi_d.ap()[:, SH0:])
            nc.sync.dma_start(out=dstl_sb[:, M0:], in_=dstl_d.ap()[:, M0:])
            nc.sync.dma_start(out=wt_sb[:, M0:], in_=wt_d.ap()[:, M0:])
            iota_sb = meta.tile([P, P], bf16)
            nc.sync.dma_start(out=iota_sb[:], in_=iota_d.ap())
            # iota lives in PSUM: the mask tensor_scalar reads it via the
            # PSUM port, keeping the DVE off the SBUF read ports that the
            # gather writes + SWDGE descriptor rings need (measured: an
            # SBUF-sourced mask stalls the gathers).
            iota_ps = ps_c.tile([P, P], bf16 if iota_bf16_ps else f32)
            nc.scalar.copy(iota_ps[:], iota_sb[:])
            if scalars_ps:
                dstl_ps = ps_c.tile([P, M], f32, tag="dstl_ps")
                nc.scalar.copy(dstl_ps[:], dstl_sb[:])
                wt_ps = ps_c.tile([P, M], f32, tag="wt_ps")
                nc.scalar.copy(wt_ps[:], wt_sb[:])
            else:
                dstl_ps, wt_ps = dstl_sb, wt_sb
            w1_sb = meta.tile([D, D], bf16)
            nc.sync.dma_start(out=w1_sb[:], in_=w1_d.ap())
            w2_sb = meta.tile([D, D], bf16)
            nc.sync.dma_start(out=w2_sb[:], in_=w2_d.ap())
            b1_sb = meta.tile([D, 1], f32)
            nc.sync.dma_start(out=b1_sb[:], in_=b1_d.ap())
            if b2_nonzero:
                b2b_sb = meta.tile([P, D], f32)
                nc.sync.dma_start(out=b2b_sb[:], in_=b2b_d.ap())

            max_C = max(C_lo_s[b] + C_hi_s[b] for b in range(NBC))
            # blocks are gathered in groups of G: one lo + one hi dma_gather
            # per group (their idx columns are contiguous across the group's
            # blocks), amortizing the per-op SWDGE fixed cost on Pool.
            groups = [list(range(g, min(g + G, NBC)))
                      for g in range(0, NBC, G)]
            max_CL = max(sum(C_lo_s[b] for b in grp) for grp in groups)
            max_CH = max(sum(C_hi_s[b] for b in grp) for grp in groups)
            gq = 0
            for _ in range(repeats):
                for grp in groups:
                    GL = [C_lo_s[b] for b in grp]
                    GH = [C_hi_s[b] for b in grp]
                    CL, CH = sum(GL), sum(GH)
                    CG = CL + CH
                    b0 = grp[0]
                    if not no_gather:
                        xg = gat.tile([P, max_CL, TW], bf16, tag="xg")
                        xh = gath.tile([P, max_CH, TW], bf16, tag="xh")
                        in_lo = xt.ap() if pack512 else xt.ap()[0:SPL, :]
                        in_hi = xt.ap() if pack512 else xt.ap()[SPL:NPAD, :]
                        nc.gpsimd.dma_gather(
                            out_ap=xg[:, 0:CL, :], in_ap=in_lo,
                            idxs_ap=ilo_sb[:, lo_off16[b0]:
                                           lo_off16[b0] + CL * 8],
                            num_idxs=CL * P, num_idxs_reg=CL * P,
                            elem_size=TW, single_packet=sp,
                            queue_num=gq % 4)
                        nc.gpsimd.dma_gather(
                            out_ap=xh[:, 0:CH, :], in_ap=in_hi,
                            idxs_ap=ihi_sb[:, hi_off16[b0]:
                                           hi_off16[b0] + CH * 8],
                            num_idxs=CH * P, num_idxs_reg=CH * P,
                            elem_size=TW, single_packet=sp,
                            queue_num=(gq + 1) % 4)
                        gq += 2
                    for j, b in enumerate(grp):
                        C_lo, C_hi = C_lo_s[b], C_hi_s[b]
                        lo_base = sum(GL[:j])
                        hi_base = sum(GH[:j])
                        agg_ps = ps_agg.tile([D, P], f32, tag="agg")
                        if stream_masks:
                            CB = C_lo + C_hi
                            mst = msk.tile([P, max_C * P], bf16, tag="mst")
                            nc.sync.dma_start(
                                out=mst[:, 0:CB * P],
                                in_=maskt_d.ap()[:, col_off[b] * P:
                                                 (col_off[b] + CB) * P])
                        for t in range(C_lo + C_hi):
                            c = col_off[b] + t
                            if no_gather:
                                src_t = iota_sb[:]
                            elif pack512:
                                src_t = xg[:, lo_base + t, 0:D] if t < C_lo \
                                    else xh[:, hi_base + (t - C_lo), D:2 * D]
                            else:
                                src_t = xg[:, lo_base + t, :] if t < C_lo \
                                    else xh[:, hi_base + (t - C_lo), :]
                            if no_mask:
                                mask_ap = iota_sb[:]
                            elif stream_masks:
                                mask_ap = mst[:, t * P:(t + 1) * P]
                            elif act_mask is not None and act_mask(t):
                                # ACT-built mask (private SBUF ports):
                                # mask = relu(w~ - w~*|iota-dstl|)
                                ad = msk.tile([P, P], bf16, tag="actm")
                                nc.scalar.activation(
                                    ad[:], iota_sb[:],
                                    mybir.ActivationFunctionType.Abs,
                                    bias=ndstl_sb[:, c:c + 1], scale=1.0)
                                mask = msk.tile([P, P], bf16, tag="mask")
                                nc.scalar.activation(
                                    mask[:], ad[:],
                                    mybir.ActivationFunctionType.Relu,
                                    bias=wt_sb[:, c:c + 1],
                                    scale=nwt_sb[:, c:c + 1])
                                mask_ap = mask[:]
                            else:
                                mask = msk.tile([P, P], bf16, tag="mask")
                                nc.vector.tensor_scalar(
                                    out=mask[:], in0=iota_ps[:],
                                    scalar1=dstl_ps[:, c:c + 1],
                                    scalar2=wt_ps[:, c:c + 1],
                                    op0=mybir.AluOpType.is_equal,
                                    op1=mybir.AluOpType.mult)
                                mask_ap = mask[:]
                            nc.tensor.matmul(out=agg_ps[:],
                                             lhsT=src_t, rhs=mask_ap,
                                             start=(t == 0),
                                             stop=(t == C_lo + C_hi - 1))
                        aggT_sb = eptp.tile([D, P], bf16, tag="aggT")
                        nc.scalar.copy(aggT_sb[:], agg_ps[:])
                        h_ps = ps_h.tile([D, P], f32, tag="h")
                        nc.tensor.matmul(out=h_ps[:], lhsT=w1_sb[:],
                                         rhs=aggT_sb[:], start=True, stop=True)
                        hT_sb = eptp.tile([D, P], bf16, tag="hT")
                        nc.scalar.activation(hT_sb[:], h_ps[:],
                                             mybir.ActivationFunctionType.Relu,
                                             bias=b1_sb[:, :1], scale=1.0)
                        o_ps = ps_o.tile([P, D], f32, tag="o")
                        nc.tensor.matmul(out=o_ps[:], lhsT=hT_sb[:],
                                         rhs=w2_sb[:], start=True, stop=True)
                        out_sb = eptp.tile([P, D], f32, tag="outsb")
                        if b2_nonzero:
                            nc.vector.tensor_tensor(out=out_sb[:], in0=o_ps[:],
                                                    in1=b2b_sb[:],
                                                    op=mybir.AluOpType.add)
                        else:
                            nc.scalar.copy(out_sb[:], o_ps[:])
                        nc.sync.dma_start(
                            out=out_d.ap()[b * P:(b + 1) * P, :],
                            in_=out_sb[:])
    nc.compile()
    _fix_swdge_queues(nc)
    return nc


def _fix_swdge_queues(nc):
    """Each SW-DMA sem lane (assigned round-robin by the tile scheduler) must
    map to exactly one SWDGE queue; derive queue from the final lane so the
    mapping stays consistent regardless of instruction reordering."""
    from concourse.tile_sem_assignment import PROC_NAME_TO_IDX

    lane_of = {PROC_NAME_TO_IDX[f"DMASW{i}"]: i for i in range(8)}
    for blk in nc.m.functions[0].blocks:
        for inst in blk.instructions:
            proc = getattr(inst, "bass_scheduled_proc", None)
            if proc in lane_of and hasattr(inst, "queue_num"):
                inst.queue_num = lane_of[proc] % 4


def _make_in_maps(xpad16, ilo16, ihi16, dstl16, wt16, mask_all,
                  W1, b1, W2, b2, b2_nonzero):
    iota = np.tile(np.arange(P, dtype=np.float32), (P, 1)).astype(np_bf16)
    in_maps = []
    for c in range(NCORES):
        m = {
            "xt": xpad16,
            "ilo": ilo16[c],
            "ihi": ihi16[c],
            "dstl": dstl16[c],
            "wt": wt16[c],
            "ndstl": -dstl16[c],
            "nwt": -wt16[c],
            **({"maskt": mask_all[c].reshape(P, -1)}
               if mask_all is not None else {}),
            "iota": iota,
            "w1": np.asarray(W1, np.float32).astype(np_bf16),
            "w2": np.asarray(W2, np.float32).astype(np_bf16),
            "b1": np.asarray(b1, np.float32).reshape(D, 1),
        }
        if b2_nonzero:
            m["b2b"] = np.tile(np.asarray(b2, np.float32).reshape(1, D), (P, 1))
        in_maps.append(m)
    return in_maps


def kernel(x, src, dst, edge_weights, W1, b1, W2, b2):
    layout, xpad16, ilo16, ihi16, dstl16, wt16, mask_all = \
        _host_pack(x, src, dst, edge_weights)
    b2_nonzero = bool(np.any(np.asarray(b2)))
    nc = _build_program(layout, b2_nonzero)
    in_maps = _make_in_maps(xpad16, ilo16, ihi16, dstl16, wt16, mask_all,
                            W1, b1, W2, b2, b2_nonzero)
    res = run_bass_kernel_spmd(nc, in_maps, core_ids=list(range(NCORES)))
    out = np.concatenate([res.results[c]["out"] for c in range(NCORES)], axis=0)
    return out[:N_NODES].astype(np.float32)

